# Initial kernel scaffold
#
"""Trainium2 Bass kernel for a GPT-2 style transformer block.

Problem: B=8, T=1024, C=768, H=12 heads, causal attention, GELU-tanh MLP.
Sharding: data-parallel over batch -- one batch element per NeuronCore,
weights replicated, no collectives.

Per-core dataflow (token tiles of 128, feature tiles of 128):
  P1  LN1 (bn_stats/bn_aggr) token-major, PE-transpose h -> hT (feature-major)
  P2a V = h @ Wv token-major, packed per head as [v | ones] (65 cols/head)
  P2b per head-pair: Q^T,K^T feature-major (W stationary, hT moving)
  P3  scores computed directly transposed on PE: S^T[s,t] = K^T.T@Q^T slices;
      exp(0.125*S) on ACT straight out of PSUM (no max-subtract needed --
      scores are O(+-15)); causal mask = multiply diagonal tile by 0/1
      triangle; att^T @ [v|ones] gives y^T and the softmax row-sums in one
      matmul; normalization via reciprocal + DMA partition-broadcast.
  P4  proj token-major (+residual), LN2 fused, PE-transpose h2 -> h2T
  P6  MLP streamed in 512-wide hidden strips; fc2 partials accumulated in
      SBUF; +residual, written out.
"""

import sys

if "/opt/trn_rl_repo" not in sys.path:
    sys.path.insert(0, "/opt/trn_rl_repo")

import numpy as np

import concourse.bacc as bacc
import concourse.mybir as mybir
import concourse.tile as tile
from concourse.bass_utils import run_bass_kernel_spmd
from concourse.masks import make_identity, make_upper_triangular

P = 128
T = 1024
C = 768
H = 12
D = 64
F = 3072
TT = T // P   # 8 token tiles
KC = C // P   # 6 feature tiles
NP = H // 2   # 6 head pairs
LN_EPS = 1e-5
f32 = mybir.dt.float32
AF = mybir.ActivationFunctionType
ALU = mybir.AluOpType

N_CORES = 8

WEIGHT_NAMES = [
    "ln1_g", "ln1_b", "w_attn", "b_attn", "w_proj", "b_proj",
    "ln2_g", "ln2_b", "w_fc1", "b_fc1", "w_fc2", "b_fc2",
]


def _layer_norm(nc, tmp, x_ap, g_b, b_b, out_h):
    """LN over the 768-wide free dim of a [128, 768] token tile."""
    stats = tmp.tile([P, 3, 6], f32, tag="lnstats")
    xv = x_ap.rearrange("p (a b) -> p a b", b=256)
    for a in range(3):
        nc.vector.bn_stats(out=stats[:, a, :], in_=xv[:, a, :])
    mv = tmp.tile([P, 2], f32, tag="lnmv")
    nc.vector.bn_aggr(out=mv[:], in_=stats[:])
    rs = tmp.tile([P, 1], f32, tag="lnrs")
    nc.scalar.activation(out=rs[:], in_=mv[:, 1:2], func=AF.Sqrt,
                         bias=LN_EPS, scale=1.0)
    rsr = tmp.tile([P, 1], f32, tag="lnrsr")
    nc.vector.reciprocal(out=rsr[:], in_=rs[:])
    nc.vector.tensor_scalar(out=out_h, in0=x_ap, scalar1=mv[:, 0:1],
                            scalar2=rsr[:], op0=ALU.subtract, op1=ALU.mult)
    nc.vector.tensor_mul(out=out_h, in0=out_h, in1=g_b)
    nc.vector.tensor_add(out=out_h, in0=out_h, in1=b_b)


def build_nc():
    nc = bacc.Bacc("TRN2", target_bir_lowering=False, debug=False)

    x_d = nc.dram_tensor("x", [T, C], f32, kind="ExternalInput").ap()
    w = {}
    shapes = {
        "ln1_g": [C], "ln1_b": [C], "w_attn": [C, 3 * C], "b_attn": [3 * C],
        "w_proj": [C, C], "b_proj": [C], "ln2_g": [C], "ln2_b": [C],
        "w_fc1": [C, F], "b_fc1": [F], "w_fc2": [F, C], "b_fc2": [C],
    }
    for name in WEIGHT_NAMES:
        w[name] = nc.dram_tensor(name, shapes[name], f32, kind="ExternalInput").ap()
    out_d = nc.dram_tensor("out", [T, C], f32, kind="ExternalOutput").ap()
    x2_d = nc.dram_tensor("x2scratch", [T, C], f32).ap()

    with tile.TileContext(nc) as tc:
        with (
            tc.tile_pool(name="const", bufs=1) as cp,
            tc.tile_pool(name="YTp", bufs=1) as YTp,
        ):
            ident = cp.tile([P, P], f32, tag="ident")
            make_identity(nc, ident[:])
            mask01 = cp.tile([P, P], f32, tag="mask01")
            make_upper_triangular(nc, mask01[:], val=1.0, diag=True)

            def bcast_const(name, src_ap):
                t = cp.tile([P, C], f32, tag=name)
                nc.gpsimd.dma_start(out=t[:], in_=src_ap.to_broadcast([P, C]))
                return t

            g1b = bcast_const("g1b", w["ln1_g"])
            b1b = bcast_const("b1b", w["ln1_b"])
            g2b = bcast_const("g2b", w["ln2_g"])
            b2b = bcast_const("b2b", w["ln2_b"])
            bvb = bcast_const("bvb", w["b_attn"][2 * C:3 * C])
            bpb = bcast_const("bpb", w["b_proj"])
            b2cb = bcast_const("b2cb", w["b_fc2"])

            bq = cp.tile([P, KC], f32, tag="bq")
            nc.sync.dma_start(out=bq[:], in_=w["b_attn"][0:C].rearrange("(m p) -> p m", p=P))
            bk = cp.tile([P, KC], f32, tag="bk")
            nc.sync.dma_start(out=bk[:], in_=w["b_attn"][C:2 * C].rearrange("(m p) -> p m", p=P))
            b1c = cp.tile([P, F // P], f32, tag="b1c")
            nc.sync.dma_start(out=b1c[:], in_=w["b_fc1"].rearrange("(m p) -> p m", p=P))

            YT = [YTp.tile([P, T], f32, tag=f"YT{k}") for k in range(KC)]

            # ---------------- Phases 1-3 ----------------
            with (
                tc.tile_pool(name="hTp", bufs=1) as hTp,
                tc.tile_pool(name="vp", bufs=1) as vp,
            ):
                hT = [hTp.tile([P, T], f32, tag=f"hT{k}") for k in range(KC)]

                # P1: LN1 + transpose
                with (
                    tc.tile_pool(name="p1", bufs=3) as p1p,
                    tc.tile_pool(name="p1t", bufs=4) as p1t,
                    tc.tile_pool(name="ps1", bufs=2, space="PSUM") as ps1,
                ):
                    for i in range(TT):
                        xt = p1p.tile([P, C], f32, tag="xt")
                        nc.sync.dma_start(out=xt[:], in_=x_d[i * P:(i + 1) * P, :])
                        h = p1p.tile([P, C], f32, tag="h")
                        _layer_norm(nc, p1t, xt[:], g1b[:], b1b[:], h[:])
                        for k in range(KC):
                            pst = ps1.tile([P, P], f32, tag="tr")
                            nc.tensor.transpose(out=pst[:], in_=h[:, k * P:(k + 1) * P],
                                                identity=ident[:])
                            nc.scalar.copy(out=hT[k][:, i * P:(i + 1) * P], in_=pst[:])

                # P2a: V token-major, packed [v|1] per head
                vts = []
                with (
                    tc.tile_pool(name="wav", bufs=1) as wavp,
                    tc.tile_pool(name="psv", bufs=2, space="PSUM") as psv,
                ):
                    wv = []
                    for k in range(KC):
                        wvt = wavp.tile([P, C], f32, tag=f"wav{k}")
                        nc.sync.dma_start(out=wvt[:], in_=w["w_attn"][k * P:(k + 1) * P, 2 * C:3 * C])
                        wv.append(wvt)
                    for i in range(TT):
                        psvt = psv.tile([P, C], f32, tag="psv")
                        for k in range(KC):
                            lhsT = hT[k][:, i * P:(i + 1) * P]
                            nc.tensor.matmul(out=psvt[:, 0:512], lhsT=lhsT,
                                             rhs=wv[k][:, 0:512],
                                             start=(k == 0), stop=(k == KC - 1))
                            nc.tensor.matmul(out=psvt[:, 512:768], lhsT=lhsT,
                                             rhs=wv[k][:, 512:768],
                                             start=(k == 0), stop=(k == KC - 1))
                        vt = vp.tile([P, H * (D + 1)], f32, tag=f"v{i}")
                        vv = vt[:].rearrange("p (h e) -> p h e", e=D + 1)
                        nc.vector.tensor_add(
                            out=vv[:, :, 0:D],
                            in0=psvt[:].rearrange("p (h e) -> p h e", e=D),
                            in1=bvb[:].rearrange("p (h e) -> p h e", e=D))
                        nc.vector.memset(vv[:, :, D:D + 1], 1.0)
                        vts.append(vt)

                # P2b/P3: per head pair QK + attention
                with (
                    tc.tile_pool(name="waqk", bufs=2) as waqkp,
                    tc.tile_pool(name="qk", bufs=2) as qkp,
                    tc.tile_pool(name="att", bufs=9) as attp,
                    tc.tile_pool(name="rsc", bufs=1) as rscp,
                    tc.tile_pool(name="yn", bufs=2) as ynp,
                    tc.tile_pool(name="psqs", bufs=2, space="PSUM") as psqs,
                    tc.tile_pool(name="psy", bufs=2, space="PSUM") as psyp,
                ):
                    for pi in range(NP):
                        wq = waqkp.tile([P, KC, 2 * P], f32, tag="waqk")
                        for k in range(KC):
                            nc.sync.dma_start(
                                out=wq[:, k, 0:P],
                                in_=w["w_attn"][k * P:(k + 1) * P, pi * P:(pi + 1) * P])
                            nc.sync.dma_start(
                                out=wq[:, k, P:2 * P],
                                in_=w["w_attn"][k * P:(k + 1) * P, C + pi * P:C + (pi + 1) * P])
                        qT = qkp.tile([P, T], f32, tag="qT")
                        kT = qkp.tile([P, T], f32, tag="kT")
                        for (dst, woff, bcol) in ((qT, 0, bq), (kT, P, bk)):
                            psq = psqs.tile([P, T], f32, tag="ps")
                            for k in range(KC):
                                lhsT = wq[:, k, woff:woff + P]
                                nc.tensor.matmul(out=psq[:, 0:512], lhsT=lhsT,
                                                 rhs=hT[k][:, 0:512],
                                                 start=(k == 0), stop=(k == KC - 1))
                                nc.tensor.matmul(out=psq[:, 512:1024], lhsT=lhsT,
                                                 rhs=hT[k][:, 512:1024],
                                                 start=(k == 0), stop=(k == KC - 1))
                            nc.scalar.activation(out=dst[:], in_=psq[:], func=AF.Identity,
                                                 bias=bcol[:, pi:pi + 1], scale=1.0)

                        for hh in (2 * pi, 2 * pi + 1):
                            off = (hh % 2) * D
                            qh = qT[off:off + D, :]
                            kh = kT[off:off + D, :]
                            atts = []
                            for j in range(TT):
                                nt = (TT - j) * P
                                pss = psqs.tile([P, T], f32, tag="ps")
                                for c0 in range(0, nt, 512):
                                    cw = min(512, nt - c0)
                                    nc.tensor.matmul(
                                        out=pss[:, c0:c0 + cw],
                                        lhsT=kh[:, j * P:(j + 1) * P],
                                        rhs=qh[:, j * P + c0:j * P + c0 + cw],
                                        start=True, stop=True)
                                at = attp.tile([P, T], f32, tag="att")
                                nc.scalar.activation(out=at[:, 0:nt], in_=pss[:, 0:nt],
                                                     func=AF.Exp, scale=0.125)
                                nc.vector.tensor_mul(out=at[:, 0:P], in0=at[:, 0:P],
                                                     in1=mask01[:])
                                atts.append(at)
                            yA = psyp.tile([D + 1, 512], f32, tag="yA")
                            yB = psyp.tile([D + 1, 512], f32, tag="yB")
                            for j in range(TT):
                                vloc = vts[j][:, hh * (D + 1):(hh + 1) * (D + 1)]
                                for i in range(j, TT):
                                    tgt = yA if i < 4 else yB
                                    col = (i % 4) * P
                                    nc.tensor.matmul(
                                        out=tgt[:, col:col + P], lhsT=vloc,
                                        rhs=atts[j][:, (i - j) * P:(i - j + 1) * P],
                                        start=(j == 0), stop=(j == i),
                                        skip_group_check=True)
                            rrow = rscp.tile([D + 1, T], f32, tag="rrow")
                            nc.vector.reciprocal(out=rrow[D:D + 1, 0:512], in_=yA[D:D + 1, :])
                            nc.vector.reciprocal(out=rrow[D:D + 1, 512:1024], in_=yB[D:D + 1, :])
                            Rsb = rscp.tile([D, T], f32, tag="Rsb")
                            nc.gpsimd.partition_broadcast(out_ap=Rsb[:], in_ap=rrow[D:D + 1, :])
                            if off == 0:
                                nc.vector.tensor_mul(out=YT[pi][0:D, 0:512],
                                                     in0=yA[0:D, :], in1=Rsb[:, 0:512])
                                nc.vector.tensor_mul(out=YT[pi][0:D, 512:1024],
                                                     in0=yB[0:D, :], in1=Rsb[:, 512:1024])
                            else:
                                ynt = ynp.tile([D, T], f32, tag="yn")
                                nc.vector.tensor_mul(out=ynt[:, 0:512],
                                                     in0=yA[0:D, :], in1=Rsb[:, 0:512])
                                nc.vector.tensor_mul(out=ynt[:, 512:1024],
                                                     in0=yB[0:D, :], in1=Rsb[:, 512:1024])
                                nc.sync.dma_start(out=YT[pi][D:P, :], in_=ynt[:])

            # ---------------- Phases 4-6 ----------------
            with tc.tile_pool(name="h2Tp", bufs=1) as h2Tp:
                h2T = [h2Tp.tile([P, T], f32, tag=f"h2T{k}") for k in range(KC)]

                with (
                    tc.tile_pool(name="wpp", bufs=1) as wpp,
                    tc.tile_pool(name="p4", bufs=3) as p4p,
                    tc.tile_pool(name="p4t", bufs=4) as p4t,
                    tc.tile_pool(name="ps4", bufs=2, space="PSUM") as ps4,
                    tc.tile_pool(name="ps4t", bufs=2, space="PSUM") as ps4t,
                ):
                    wps = []
                    for k in range(KC):
                        wpt = wpp.tile([P, C], f32, tag=f"wp{k}")
                        nc.sync.dma_start(out=wpt[:], in_=w["w_proj"][k * P:(k + 1) * P, :])
                        wps.append(wpt)
                    for i in range(TT):
                        xre = p4p.tile([P, C], f32, tag="xre")
                        nc.sync.dma_start(out=xre[:], in_=x_d[i * P:(i + 1) * P, :])
                        psp = ps4.tile([P, C], f32, tag="psp")
                        for k in range(KC):
                            lhsT = YT[k][:, i * P:(i + 1) * P]
                            nc.tensor.matmul(out=psp[:, 0:512], lhsT=lhsT,
                                             rhs=wps[k][:, 0:512],
                                             start=(k == 0), stop=(k == KC - 1))
                            nc.tensor.matmul(out=psp[:, 512:768], lhsT=lhsT,
                                             rhs=wps[k][:, 512:768],
                                             start=(k == 0), stop=(k == KC - 1))
                        x2 = p4p.tile([P, C], f32, tag="x2")
                        nc.vector.tensor_add(out=x2[:], in0=psp[:], in1=xre[:])
                        nc.vector.tensor_add(out=x2[:], in0=x2[:], in1=bpb[:])
                        nc.sync.dma_start(out=x2_d[i * P:(i + 1) * P, :], in_=x2[:])
                        h2 = p4p.tile([P, C], f32, tag="h2")
                        _layer_norm(nc, p4t, x2[:], g2b[:], b2b[:], h2[:])
                        for k in range(KC):
                            pst = ps4t.tile([P, P], f32, tag="tr2")
                            nc.tensor.transpose(out=pst[:], in_=h2[:, k * P:(k + 1) * P],
                                                identity=ident[:])
                            nc.scalar.copy(out=h2T[k][:, i * P:(i + 1) * P], in_=pst[:])

                # P6: MLP in 512-wide hidden strips
                with (
                    tc.tile_pool(name="mw", bufs=2) as mwp,
                    tc.tile_pool(name="gt", bufs=5) as gtp,
                    tc.tile_pool(name="accp", bufs=1) as accp,
                    tc.tile_pool(name="x2r", bufs=3) as x2rp,
                    tc.tile_pool(name="psg", bufs=2, space="PSUM") as psg,
                    tc.tile_pool(name="psf", bufs=2, space="PSUM") as psf,
                ):
                    accs = [accp.tile([P, C], f32, tag=f"acc{i}") for i in range(TT)]
                    for s in range(F // 512):
                        w1s = []
                        for k in range(KC):
                            w1t = mwp.tile([P, 512], f32, tag=f"w1_{k}")
                            nc.sync.dma_start(out=w1t[:],
                                              in_=w["w_fc1"][k * P:(k + 1) * P, s * 512:(s + 1) * 512])
                            w1s.append(w1t)
                        w2s = []
                        for kk in range(4):
                            w2t = mwp.tile([P, C], f32, tag=f"w2_{kk}")
                            nc.sync.dma_start(out=w2t[:],
                                              in_=w["w_fc2"][(s * 4 + kk) * P:(s * 4 + kk + 1) * P, :])
                            w2s.append(w2t)
                        gts = []
                        for m in range(4):
                            psgt = psg.tile([P, T], f32, tag="psg")
                            for k in range(KC):
                                lhsT = w1s[k][:, m * P:(m + 1) * P]
                                nc.tensor.matmul(out=psgt[:, 0:512], lhsT=lhsT,
                                                 rhs=h2T[k][:, 0:512],
                                                 start=(k == 0), stop=(k == KC - 1))
                                nc.tensor.matmul(out=psgt[:, 512:1024], lhsT=lhsT,
                                                 rhs=h2T[k][:, 512:1024],
                                                 start=(k == 0), stop=(k == KC - 1))
                            gt = gtp.tile([P, T], f32, tag="gt")
                            nc.scalar.activation(out=gt[:], in_=psgt[:],
                                                 func=AF.Gelu_apprx_tanh,
                                                 bias=b1c[:, s * 4 + m:s * 4 + m + 1],
                                                 scale=1.0)
                            gts.append(gt)
                        for i in range(TT):
                            psft = psf.tile([P, C], f32, tag="psf")
                            for kk in range(4):
                                lhsT = gts[kk][:, i * P:(i + 1) * P]
                                nc.tensor.matmul(out=psft[:, 0:512], lhsT=lhsT,
                                                 rhs=w2s[kk][:, 0:512],
                                                 start=(kk == 0), stop=(kk == 3))
                                nc.tensor.matmul(out=psft[:, 512:768], lhsT=lhsT,
                                                 rhs=w2s[kk][:, 512:768],
                                                 start=(kk == 0), stop=(kk == 3))
                            if s == 0:
                                x2r = x2rp.tile([P, C], f32, tag="x2r")
                                nc.sync.dma_start(out=x2r[:], in_=x2_d[i * P:(i + 1) * P, :])
                                nc.vector.tensor_add(out=accs[i][:], in0=psft[:], in1=x2r[:])
                                nc.vector.tensor_add(out=accs[i][:], in0=accs[i][:], in1=b2cb[:])
                            else:
                                nc.vector.tensor_add(out=accs[i][:], in0=accs[i][:], in1=psft[:])
                            if s == F // 512 - 1:
                                nc.sync.dma_start(out=out_d[i * P:(i + 1) * P, :], in_=accs[i][:])

    nc.compile()
    return nc


_NC_CACHE = {}


def _get_nc():
    if "nc" not in _NC_CACHE:
        _NC_CACHE["nc"] = build_nc()
    return _NC_CACHE["nc"]


def kernel(**inputs):
    x = np.ascontiguousarray(np.asarray(inputs["x"], dtype=np.float32))
    assert x.shape == (N_CORES, T, C), x.shape
    weights = {n: np.ascontiguousarray(np.asarray(inputs[n], dtype=np.float32))
               for n in WEIGHT_NAMES}
    nc = _get_nc()
    in_maps = []
    for c in range(N_CORES):
        m = {"x": np.ascontiguousarray(x[c])}
        m.update(weights)
        in_maps.append(m)
    res = run_bass_kernel_spmd(nc, in_maps, core_ids=list(range(N_CORES)))
    return np.stack([np.asarray(res.results[c]["out"]) for c in range(N_CORES)], axis=0)


if __name__ == "__main__":
    rng = np.random.default_rng(0)
    ins = {
        "x": rng.standard_normal((N_CORES, T, C), dtype=np.float32),
        "ln1_g": np.ones(C, np.float32), "ln1_b": np.zeros(C, np.float32),
        "w_attn": rng.standard_normal((C, 3 * C), dtype=np.float32) * 0.02,
        "b_attn": np.zeros(3 * C, np.float32),
        "w_proj": rng.standard_normal((C, C), dtype=np.float32) * 0.02,
        "b_proj": np.zeros(C, np.float32),
        "ln2_g": np.ones(C, np.float32), "ln2_b": np.zeros(C, np.float32),
        "w_fc1": rng.standard_normal((C, F), dtype=np.float32) * 0.02,
        "b_fc1": np.zeros(F, np.float32),
        "w_fc2": rng.standard_normal((F, C), dtype=np.float32) * 0.02,
        "b_fc2": np.zeros(C, np.float32),
    }
    out = kernel(**ins)
    print("out", out.shape, out.dtype, float(np.abs(out).max()))


# revision 17
# speedup vs baseline: 1.3391x; 1.3391x over previous
"""Trainium2 Bass kernel for a GPT-2 style transformer block.

Problem: B=8, T=1024, C=768, H=12 heads, causal attention, GELU-tanh MLP.
Sharding: data-parallel over batch -- one batch element per NeuronCore,
weights replicated, no collectives.

Per-core dataflow (token tiles of 128, feature tiles of 128):
  P1  LN1 (bn_stats/bn_aggr, fp32) token-major, PE-transpose h -> hT
      (feature-major, stored bf16)
  P2a V = h @ Wv token-major, packed per head as [v | ones] (65 cols/head)
  P2b per head-pair: Q^T,K^T feature-major (W stationary, hT moving)
  P3  scores computed directly transposed on PE: S^T[s,t] = K^T.T@Q^T;
      exp(0.125*S) on ACT straight out of PSUM (no max-subtract needed --
      scores are O(+-15)); causal mask = multiply diagonal tile by 0/1
      triangle; att^T @ [v|ones] gives y^T and the softmax row-sums in the
      same matmuls; per-t normalization via reciprocal + PE ones-broadcast.
  P4  proj token-major (+residual fp32), LN2 fused, PE-transpose h2 -> h2T
  P6  MLP streamed in 512-wide hidden strips; fc2 partials accumulated in
      SBUF fp32; +residual, written out.

Matmul operands are bf16 (fp32 PSUM accumulation); LN statistics,
residual stream, softmax reciprocals and all bias adds stay fp32.
"""

import sys

if "/opt/trn_rl_repo" not in sys.path:
    sys.path.insert(0, "/opt/trn_rl_repo")

import ml_dtypes
import numpy as np

import concourse.bass as bass
import concourse.bacc as bacc
import concourse.mybir as mybir
import concourse.tile as tile
from concourse.bass_utils import run_bass_kernel_spmd
from concourse.masks import make_identity, make_upper_triangular

P = 128
T = 1024
C = 768
H = 12
D = 64
F = 3072
TT = T // P   # 8 token tiles
KC = C // P   # 6 feature tiles
NP = H // 2   # 6 head pairs
LN_EPS = 1e-5
f32 = mybir.dt.float32
bf16 = mybir.dt.bfloat16
AF = mybir.ActivationFunctionType
ALU = mybir.AluOpType

N_CORES = 8

WEIGHT_NAMES = [
    "ln1_g", "ln1_b", "w_attn", "b_attn", "w_proj", "b_proj",
    "ln2_g", "ln2_b", "w_fc1", "b_fc1", "w_fc2", "b_fc2",
]
BF16_NAMES = {"w_attn", "w_proj", "w_fc1", "w_fc2"}


def _layer_norm(nc, tmp, x_ap, g_b, b_b, out_h, eps_ap):
    """LN over the 768-wide free dim of a [128, 768] token tile (fp32)."""
    stats = tmp.tile([P, 3, 6], f32, tag="lnstats")
    xv = x_ap.rearrange("p (a b) -> p a b", b=256)
    for a in range(3):
        nc.vector.bn_stats(out=stats[:, a, :], in_=xv[:, a, :])
    mv = tmp.tile([P, 2], f32, tag="lnmv")
    nc.vector.bn_aggr(out=mv[:], in_=stats[:])
    rs = tmp.tile([P, 1], f32, tag="lnrs")
    nc.scalar.activation(out=rs[:], in_=mv[:, 1:2], func=AF.Sqrt,
                         bias=eps_ap, scale=1.0)
    rsr = tmp.tile([P, 1], f32, tag="lnrsr")
    nc.vector.reciprocal(out=rsr[:], in_=rs[:])
    nc.vector.tensor_scalar(out=out_h, in0=x_ap, scalar1=mv[:, 0:1],
                            scalar2=rsr[:], op0=ALU.subtract, op1=ALU.mult)
    nc.vector.tensor_mul(out=out_h, in0=out_h, in1=g_b)
    nc.vector.tensor_add(out=out_h, in0=out_h, in1=b_b)


def build_nc(sim_safe_gelu=False, debug_dump=False, n_copies=1):
    nc = bacc.Bacc("TRN2", target_bir_lowering=False, debug=False)

    x_d = nc.dram_tensor("x", [T, C], f32, kind="ExternalInput").ap()
    w = {}
    shapes = {
        "ln1_g": [C], "ln1_b": [C], "w_attn": [C, 3 * C], "b_attn": [3 * C],
        "w_proj": [C, C], "b_proj": [C], "ln2_g": [C], "ln2_b": [C],
        "w_fc1": [C, F], "b_fc1": [F], "w_fc2": [F, C], "b_fc2": [C],
    }
    for name in WEIGHT_NAMES:
        dt = bf16 if name in BF16_NAMES else f32
        w[name] = nc.dram_tensor(name, shapes[name], dt, kind="ExternalInput").ap()
    out_d = nc.dram_tensor("out", [T, C], f32, kind="ExternalOutput").ap()
    x2_d = nc.dram_tensor("x2scratch", [T, C], f32).ap()
    dbg = {}
    if debug_dump:
        for nm, shp, dt in [("dbg_h", [T, C], f32), ("dbg_hT", [C, T], bf16),
                            ("dbg_qT", [C, T], bf16), ("dbg_kT", [C, T], bf16),
                            ("dbg_v", [T, H * (D + 1)], bf16),
                            ("dbg_att0", [T, T], bf16), ("dbg_YT", [C, T], bf16),
                            ("dbg_h2T", [C, T], bf16), ("dbg_R", [2 * H, T], f32)]:
            dbg[nm] = nc.dram_tensor(nm, shp, dt, kind="ExternalOutput").ap()

    with tile.TileContext(nc) as tc:
        for _rep in range(n_copies):
            with (
                tc.tile_pool(name="const", bufs=1) as cp,
                tc.tile_pool(name="YTp", bufs=1) as YTp,
            ):
                ident = cp.tile([P, P], f32, tag="ident")
                make_identity(nc, ident[:])
                mask01 = cp.tile([P, P], bf16, tag="mask01")
                make_upper_triangular(nc, mask01[:], val=1.0, diag=True)
                epsc = cp.tile([P, 1], f32, tag="epsc")
                nc.vector.memset(epsc[:], LN_EPS)
                ones_c = cp.tile([P, D], bf16, tag="ones_c")
                nc.vector.memset(ones_c[:], 1.0)

                def bcast_const(name, src_ap):
                    t = cp.tile([P, C], f32, tag=name)
                    bc = bass.AP(tensor=src_ap.tensor, offset=src_ap.offset,
                                 ap=[[0, P]] + list(src_ap.ap))
                    nc.gpsimd.dma_start(out=t[:], in_=bc)
                    return t

                g1b = bcast_const("g1b", w["ln1_g"])
                b1b = bcast_const("b1b", w["ln1_b"])
                g2b = bcast_const("g2b", w["ln2_g"])
                b2b = bcast_const("b2b", w["ln2_b"])
                bvb = bcast_const("bvb", w["b_attn"][2 * C:3 * C])
                bpb = bcast_const("bpb", w["b_proj"])
                b2cb = bcast_const("b2cb", w["b_fc2"])

                bq = cp.tile([P, KC], f32, tag="bq")
                nc.sync.dma_start(out=bq[:], in_=w["b_attn"][0:C].rearrange("(m p) -> p m", p=P))
                bk = cp.tile([P, KC], f32, tag="bk")
                nc.sync.dma_start(out=bk[:], in_=w["b_attn"][C:2 * C].rearrange("(m p) -> p m", p=P))
                b1c = cp.tile([P, F // P], f32, tag="b1c")
                nc.sync.dma_start(out=b1c[:], in_=w["b_fc1"].rearrange("(m p) -> p m", p=P))

                YT = [YTp.tile([P, T], bf16, tag=f"YT{k}", name=f"YT{k}") for k in range(KC)]

                # ---------------- Phases 1-3 ----------------
                with (
                    tc.tile_pool(name="hTp", bufs=1) as hTp,
                    tc.tile_pool(name="vp", bufs=1) as vp,
                ):
                    hT = [hTp.tile([P, T], bf16, tag=f"hT{k}", name=f"hT{k}") for k in range(KC)]

                    # P1: LN1 + transpose
                    with (
                        tc.tile_pool(name="p1", bufs=3) as p1p,
                        tc.tile_pool(name="p1t", bufs=4) as p1t,
                        tc.tile_pool(name="ps1", bufs=3, space="PSUM") as ps1,
                    ):
                        for i in range(TT):
                            xt = p1p.tile([P, C], f32, tag="xt")
                            nc.sync.dma_start(out=xt[:], in_=x_d[i * P:(i + 1) * P, :])
                            h = p1p.tile([P, C], f32, tag="h")
                            _layer_norm(nc, p1t, xt[:], g1b[:], b1b[:], h[:], epsc[:])
                            for k in range(KC):
                                pst = ps1.tile([P, P], f32, tag="tr")
                                nc.tensor.transpose(out=pst[:], in_=h[:, k * P:(k + 1) * P],
                                                    identity=ident[:])
                                nc.scalar.copy(out=hT[k][:, i * P:(i + 1) * P], in_=pst[:])
                            if debug_dump:
                                nc.sync.dma_start(out=dbg["dbg_h"][i * P:(i + 1) * P, :], in_=h[:])
                        if debug_dump:
                            for k in range(KC):
                                nc.sync.dma_start(out=dbg["dbg_hT"][k * P:(k + 1) * P, :], in_=hT[k][:])

                    # P2a: V token-major, packed [v|1] per head
                    vts = []
                    with (
                        tc.tile_pool(name="wav", bufs=1) as wavp,
                        tc.tile_pool(name="psv", bufs=2, space="PSUM") as psv,
                    ):
                        wv = []
                        for k in range(KC):
                            wvt = wavp.tile([P, C], bf16, tag=f"wav{k}")
                            nc.sync.dma_start(out=wvt[:], in_=w["w_attn"][k * P:(k + 1) * P, 2 * C:3 * C])
                            wv.append(wvt)
                        for i in range(TT):
                            psvt = psv.tile([P, C], f32, tag="psv")
                            for k in range(KC):
                                lhsT = hT[k][:, i * P:(i + 1) * P]
                                nc.tensor.matmul(out=psvt[:, 0:512], lhsT=lhsT,
                                                 rhs=wv[k][:, 0:512],
                                                 start=(k == 0), stop=(k == KC - 1))
                                nc.tensor.matmul(out=psvt[:, 512:768], lhsT=lhsT,
                                                 rhs=wv[k][:, 512:768],
                                                 start=(k == 0), stop=(k == KC - 1))
                            vt = vp.tile([P, H * (D + 1)], bf16, tag=f"v{i}")
                            vv = vt[:].rearrange("p (h e) -> p h e", e=D + 1)
                            nc.vector.tensor_add(
                                out=vv[:, :, 0:D],
                                in0=psvt[:].rearrange("p (h e) -> p h e", e=D),
                                in1=bvb[:].rearrange("p (h e) -> p h e", e=D))
                            nc.vector.memset(vv[:, :, D:D + 1], 1.0)
                            vts.append(vt)
                            if debug_dump:
                                nc.sync.dma_start(out=dbg["dbg_v"][i * P:(i + 1) * P, :], in_=vt[:])

                    # P2b/P3: per head pair QK + attention
                    with (
                        tc.tile_pool(name="waqk", bufs=2) as waqkp,
                        tc.tile_pool(name="qk", bufs=2) as qkp,
                        tc.tile_pool(name="att", bufs=9) as attp,
                        tc.tile_pool(name="rsc", bufs=2) as rscp,
                        tc.tile_pool(name="yn", bufs=2) as ynp,
                        tc.tile_pool(name="psqs", bufs=3, space="PSUM") as psqs,
                        tc.tile_pool(name="psy", bufs=1, space="PSUM") as psyp,
                    ):
                        for pi in range(NP):
                            wq = waqkp.tile([P, KC, 2 * P], bf16, tag="waqk")
                            for k in range(KC):
                                nc.sync.dma_start(
                                    out=wq[:, k, 0:P],
                                    in_=w["w_attn"][k * P:(k + 1) * P, pi * P:(pi + 1) * P])
                                nc.sync.dma_start(
                                    out=wq[:, k, P:2 * P],
                                    in_=w["w_attn"][k * P:(k + 1) * P, C + pi * P:C + (pi + 1) * P])
                            qT = qkp.tile([P, T], bf16, tag="qT")
                            kT = qkp.tile([P, T], bf16, tag="kT")
                            for (dst, woff, bcol) in ((qT, 0, bq), (kT, P, bk)):
                                psq = psqs.tile([P, T], f32, tag="ps")
                                for k in range(KC):
                                    lhsT = wq[:, k, woff:woff + P]
                                    nc.tensor.matmul(out=psq[:, 0:512], lhsT=lhsT,
                                                     rhs=hT[k][:, 0:512],
                                                     start=(k == 0), stop=(k == KC - 1))
                                    nc.tensor.matmul(out=psq[:, 512:1024], lhsT=lhsT,
                                                     rhs=hT[k][:, 512:1024],
                                                     start=(k == 0), stop=(k == KC - 1))
                                nc.scalar.activation(out=dst[:], in_=psq[:], func=AF.Identity,
                                                     bias=bcol[:, pi:pi + 1], scale=1.0)
                            if debug_dump:
                                nc.sync.dma_start(out=dbg["dbg_qT"][pi * P:(pi + 1) * P, :], in_=qT[:])
                                nc.sync.dma_start(out=dbg["dbg_kT"][pi * P:(pi + 1) * P, :], in_=kT[:])

                            for hh in (2 * pi, 2 * pi + 1):
                                off = (hh % 2) * D
                                qh = qT[off:off + D, :]
                                kh = kT[off:off + D, :]
                                atts = []
                                for j in range(TT):
                                    nt = (TT - j) * P
                                    pss = psqs.tile([P, T], f32, tag="ps")
                                    for c0 in range(0, nt, 512):
                                        cw = min(512, nt - c0)
                                        nc.tensor.matmul(
                                            out=pss[:, c0:c0 + cw],
                                            lhsT=kh[:, j * P:(j + 1) * P],
                                            rhs=qh[:, j * P + c0:j * P + c0 + cw],
                                            start=True, stop=True)
                                    at = attp.tile([P, T], bf16, tag="att")
                                    nc.scalar.activation(out=at[:, 0:nt], in_=pss[:, 0:nt],
                                                         func=AF.Exp, scale=0.125)
                                    nc.vector.tensor_mul(out=at[:, 0:P], in0=at[:, 0:P],
                                                         in1=mask01[:])
                                    atts.append(at)
                                    if debug_dump and hh == 0:
                                        nc.sync.dma_start(
                                            out=dbg["dbg_att0"][j * P:(j + 1) * P, j * P:T],
                                            in_=at[:, 0:nt])
                                # att^T @ [v|1]: merged matmuls, one per (pass, j).
                                # Pass A covers t-tiles 0..3 (yA), pass B 4..7 (yB).
                                yA = psyp.tile([D + 1, 512], f32, tag="yA")
                                yB = psyp.tile([D + 1, 512], f32, tag="yB")
                                for j in range(4):
                                    vloc = vts[j][:, hh * (D + 1):(hh + 1) * (D + 1)]
                                    nc.tensor.matmul(
                                        out=yA[:, j * P:512], lhsT=vloc,
                                        rhs=atts[j][:, 0:(4 - j) * P],
                                        start=(j == 0), stop=(j == 3))
                                for j in range(TT):
                                    vloc = vts[j][:, hh * (D + 1):(hh + 1) * (D + 1)]
                                    c0 = max(j - 4, 0) * P
                                    r0 = (max(j, 4) - j) * P
                                    nc.tensor.matmul(
                                        out=yB[:, c0:512], lhsT=vloc,
                                        rhs=atts[j][:, r0:(TT - j) * P],
                                        start=(j == 0), stop=(j == TT - 1))
                                rrow = rscp.tile([D + 1, T], f32, tag="rrow")
                                nc.vector.reciprocal(out=rrow[D:D + 1, 0:512], in_=yA[D:D + 1, :])
                                nc.vector.reciprocal(out=rrow[D:D + 1, 512:1024], in_=yB[D:D + 1, :])
                                rbf = rscp.tile([D + 1, T], bf16, tag="rbf")
                                nc.vector.tensor_copy(out=rbf[D:D + 1, :], in_=rrow[D:D + 1, :])
                                psR = psqs.tile([P, T], f32, tag="ps", name="psR")
                                nc.tensor.matmul(out=psR[0:D, 0:512],
                                                 lhsT=ones_c[D:D + 1, :],
                                                 rhs=rbf[D:D + 1, 0:512],
                                                 start=True, stop=True)
                                nc.tensor.matmul(out=psR[0:D, 512:1024],
                                                 lhsT=ones_c[D:D + 1, :],
                                                 rhs=rbf[D:D + 1, 512:1024],
                                                 start=True, stop=True)
                                Rsb = rscp.tile([D, T], f32, tag="Rsb")
                                nc.scalar.copy(out=Rsb[:], in_=psR[0:D, :])
                                if debug_dump:
                                    nc.sync.dma_start(out=dbg["dbg_R"][2 * hh:2 * hh + 1, :],
                                                      in_=Rsb[0:1, :])
                                    nc.sync.dma_start(out=dbg["dbg_R"][2 * hh + 1:2 * hh + 2, :],
                                                      in_=Rsb[D - 1:D, :])
                                if off == 0:
                                    nc.vector.tensor_mul(out=YT[pi][0:D, 0:512],
                                                         in0=yA[0:D, :], in1=Rsb[:, 0:512])
                                    nc.vector.tensor_mul(out=YT[pi][0:D, 512:1024],
                                                         in0=yB[0:D, :], in1=Rsb[:, 512:1024])
                                else:
                                    ynt = ynp.tile([D, T], bf16, tag="yn")
                                    nc.vector.tensor_mul(out=ynt[:, 0:512],
                                                         in0=yA[0:D, :], in1=Rsb[:, 0:512])
                                    nc.vector.tensor_mul(out=ynt[:, 512:1024],
                                                         in0=yB[0:D, :], in1=Rsb[:, 512:1024])
                                    nc.sync.dma_start(out=YT[pi][D:P, :], in_=ynt[:])

                if debug_dump:
                    for k in range(KC):
                        nc.sync.dma_start(out=dbg["dbg_YT"][k * P:(k + 1) * P, :], in_=YT[k][:])

                # ---------------- Phases 4-6 ----------------
                with tc.tile_pool(name="h2Tp", bufs=1) as h2Tp:
                    h2T = [h2Tp.tile([P, T], bf16, tag=f"h2T{k}", name=f"h2T{k}") for k in range(KC)]

                    with (
                        tc.tile_pool(name="wpp", bufs=1) as wpp,
                        tc.tile_pool(name="p4", bufs=3) as p4p,
                        tc.tile_pool(name="p4t", bufs=4) as p4t,
                        tc.tile_pool(name="ps4", bufs=2, space="PSUM") as ps4,
                        tc.tile_pool(name="ps4t", bufs=3, space="PSUM") as ps4t,
                    ):
                        wps = []
                        for k in range(KC):
                            wpt = wpp.tile([P, C], bf16, tag=f"wp{k}")
                            nc.sync.dma_start(out=wpt[:], in_=w["w_proj"][k * P:(k + 1) * P, :])
                            wps.append(wpt)
                        for i in range(TT):
                            xre = p4p.tile([P, C], f32, tag="xre")
                            nc.sync.dma_start(out=xre[:], in_=x_d[i * P:(i + 1) * P, :])
                            psp = ps4.tile([P, C], f32, tag="psp")
                            for k in range(KC):
                                lhsT = YT[k][:, i * P:(i + 1) * P]
                                nc.tensor.matmul(out=psp[:, 0:512], lhsT=lhsT,
                                                 rhs=wps[k][:, 0:512],
                                                 start=(k == 0), stop=(k == KC - 1))
                                nc.tensor.matmul(out=psp[:, 512:768], lhsT=lhsT,
                                                 rhs=wps[k][:, 512:768],
                                                 start=(k == 0), stop=(k == KC - 1))
                            x2 = p4p.tile([P, C], f32, tag="x2")
                            nc.vector.tensor_add(out=x2[:], in0=psp[:], in1=xre[:])
                            nc.vector.tensor_add(out=x2[:], in0=x2[:], in1=bpb[:])
                            nc.sync.dma_start(out=x2_d[i * P:(i + 1) * P, :], in_=x2[:])
                            h2 = p4p.tile([P, C], f32, tag="h2")
                            _layer_norm(nc, p4t, x2[:], g2b[:], b2b[:], h2[:], epsc[:])
                            for k in range(KC):
                                pst = ps4t.tile([P, P], f32, tag="tr2")
                                nc.tensor.transpose(out=pst[:], in_=h2[:, k * P:(k + 1) * P],
                                                    identity=ident[:])
                                nc.scalar.copy(out=h2T[k][:, i * P:(i + 1) * P], in_=pst[:])
                        if debug_dump:
                            for k in range(KC):
                                nc.sync.dma_start(out=dbg["dbg_h2T"][k * P:(k + 1) * P, :], in_=h2T[k][:])

                    # P6: MLP in 512-wide hidden strips
                    with (
                        tc.tile_pool(name="mw", bufs=2) as mwp,
                        tc.tile_pool(name="gt", bufs=5) as gtp,
                        tc.tile_pool(name="accp", bufs=1) as accp,
                        tc.tile_pool(name="x2r", bufs=3) as x2rp,
                        tc.tile_pool(name="psg", bufs=2, space="PSUM") as psg,
                        tc.tile_pool(name="psf", bufs=2, space="PSUM") as psf,
                    ):
                        accs = [accp.tile([P, C], f32, tag=f"acc{i}", name=f"acc{i}") for i in range(TT)]
                        for s in range(F // 512):
                            w1s = []
                            for k in range(KC):
                                w1t = mwp.tile([P, 512], bf16, tag=f"w1_{k}")
                                nc.sync.dma_start(out=w1t[:],
                                                  in_=w["w_fc1"][k * P:(k + 1) * P, s * 512:(s + 1) * 512])
                                w1s.append(w1t)
                            w2s = []
                            for kk in range(4):
                                w2t = mwp.tile([P, C], bf16, tag=f"w2_{kk}")
                                nc.sync.dma_start(out=w2t[:],
                                                  in_=w["w_fc2"][(s * 4 + kk) * P:(s * 4 + kk + 1) * P, :])
                                w2s.append(w2t)
                            gts = []
                            for m in range(4):
                                psgt = psg.tile([P, T], f32, tag="psg")
                                for k in range(KC):
                                    lhsT = w1s[k][:, m * P:(m + 1) * P]
                                    nc.tensor.matmul(out=psgt[:, 0:512], lhsT=lhsT,
                                                     rhs=h2T[k][:, 0:512],
                                                     start=(k == 0), stop=(k == KC - 1))
                                    nc.tensor.matmul(out=psgt[:, 512:1024], lhsT=lhsT,
                                                     rhs=h2T[k][:, 512:1024],
                                                     start=(k == 0), stop=(k == KC - 1))
                                gt = gtp.tile([P, T], bf16, tag="gt")
                                if not sim_safe_gelu:
                                    nc.scalar.activation(out=gt[:], in_=psgt[:],
                                                         func=AF.Gelu_apprx_tanh,
                                                         bias=b1c[:, s * 4 + m:s * 4 + m + 1],
                                                         scale=1.0)
                                else:
                                    a = gtp.tile([P, T], f32, tag="ga", bufs=2)
                                    nc.scalar.activation(out=a[:], in_=psgt[:],
                                                         func=AF.Identity,
                                                         bias=b1c[:, s * 4 + m:s * 4 + m + 1],
                                                         scale=1.0)
                                    sq = gtp.tile([P, T], f32, tag="gsq", bufs=2)
                                    nc.scalar.activation(out=sq[:], in_=a[:], func=AF.Square)
                                    nc.scalar.activation(out=sq[:], in_=sq[:], func=AF.Identity,
                                                         bias=1.0, scale=0.044715)
                                    nc.vector.tensor_mul(out=sq[:], in0=sq[:], in1=a[:])
                                    nc.scalar.activation(out=sq[:], in_=sq[:], func=AF.Tanh,
                                                         scale=0.7978845608028654)
                                    nc.scalar.activation(out=sq[:], in_=sq[:], func=AF.Identity,
                                                         bias=1.0, scale=1.0)
                                    nc.vector.tensor_mul(out=gt[:], in0=sq[:], in1=a[:])
                                    nc.scalar.mul(out=gt[:], in_=gt[:], mul=0.5)
                                gts.append(gt)
                            for i in range(TT):
                                psft = psf.tile([P, C], f32, tag="psf")
                                for kk in range(4):
                                    lhsT = gts[kk][:, i * P:(i + 1) * P]
                                    nc.tensor.matmul(out=psft[:, 0:512], lhsT=lhsT,
                                                     rhs=w2s[kk][:, 0:512],
                                                     start=(kk == 0), stop=(kk == 3))
                                    nc.tensor.matmul(out=psft[:, 512:768], lhsT=lhsT,
                                                     rhs=w2s[kk][:, 512:768],
                                                     start=(kk == 0), stop=(kk == 3))
                                if s == 0:
                                    x2r = x2rp.tile([P, C], f32, tag="x2r")
                                    nc.sync.dma_start(out=x2r[:], in_=x2_d[i * P:(i + 1) * P, :])
                                    nc.vector.tensor_add(out=accs[i][:], in0=psft[:], in1=x2r[:])
                                    nc.vector.tensor_add(out=accs[i][:], in0=accs[i][:], in1=b2cb[:])
                                else:
                                    nc.vector.tensor_add(out=accs[i][:], in0=accs[i][:], in1=psft[:])
                                if s == F // 512 - 1:
                                    nc.sync.dma_start(out=out_d[i * P:(i + 1) * P, :], in_=accs[i][:])

    nc.compile()
    return nc


_NC_CACHE = {}


def _get_nc():
    if "nc" not in _NC_CACHE:
        _NC_CACHE["nc"] = build_nc()
    return _NC_CACHE["nc"]


def kernel(**inputs):
    x = np.ascontiguousarray(np.asarray(inputs["x"], dtype=np.float32))
    assert x.shape == (N_CORES, T, C), x.shape
    weights = {}
    for n in WEIGHT_NAMES:
        a = np.asarray(inputs[n], dtype=np.float32)
        if n in BF16_NAMES:
            a = a.astype(ml_dtypes.bfloat16)
        weights[n] = np.ascontiguousarray(a)
    nc = _get_nc()
    in_maps = []
    for c in range(N_CORES):
        m = {"x": np.ascontiguousarray(x[c])}
        m.update(weights)
        in_maps.append(m)
    res = run_bass_kernel_spmd(nc, in_maps, core_ids=list(range(N_CORES)))
    return np.stack([np.asarray(res.results[c]["out"]) for c in range(N_CORES)], axis=0)


if __name__ == "__main__":
    rng = np.random.default_rng(0)
    ins = {
        "x": rng.standard_normal((N_CORES, T, C), dtype=np.float32),
        "ln1_g": np.ones(C, np.float32), "ln1_b": np.zeros(C, np.float32),
        "w_attn": rng.standard_normal((C, 3 * C), dtype=np.float32) * 0.02,
        "b_attn": np.zeros(3 * C, np.float32),
        "w_proj": rng.standard_normal((C, C), dtype=np.float32) * 0.02,
        "b_proj": np.zeros(C, np.float32),
        "ln2_g": np.ones(C, np.float32), "ln2_b": np.zeros(C, np.float32),
        "w_fc1": rng.standard_normal((C, F), dtype=np.float32) * 0.02,
        "b_fc1": np.zeros(F, np.float32),
        "w_fc2": rng.standard_normal((F, C), dtype=np.float32) * 0.02,
        "b_fc2": np.zeros(C, np.float32),
    }
    out = kernel(**ins)
    print("out", out.shape, out.dtype, float(np.abs(out).max()))


# revision 19
# speedup vs baseline: 2.0597x; 1.5381x over previous
"""Trainium2 Bass kernel for a GPT-2 style transformer block.

Problem: B=8, T=1024, C=768, H=12 heads, causal attention, GELU-tanh MLP.
Sharding: data-parallel over batch -- one batch element per NeuronCore,
weights replicated, no collectives.

Per-core dataflow (token tiles of 128, feature tiles of 128):
  P1  LN1 (bn_stats/bn_aggr, fp32) token-major, PE-transpose h -> hT
      (feature-major, stored bf16)
  P2a V = h @ Wv token-major, packed per head as [v | ones] (65 cols/head)
  P2b per head-pair: Q^T,K^T feature-major (W stationary, hT moving)
  P3  scores computed directly transposed on PE: S^T[s,t] = K^T.T@Q^T;
      exp(0.125*S) on ACT straight out of PSUM (no max-subtract needed --
      scores are O(+-15)); causal mask = multiply diagonal tile by 0/1
      triangle; att^T @ [v|ones] gives y^T and the softmax row-sums in the
      same matmuls; per-t normalization via reciprocal + PE ones-broadcast.
  P4  proj token-major (+residual fp32), LN2 fused, PE-transpose h2 -> h2T
  P6  MLP streamed in 512-wide hidden strips; fc2 partials accumulated in
      SBUF fp32; +residual, written out.

Matmul operands are bf16 (fp32 PSUM accumulation); LN statistics,
residual stream, softmax reciprocals and all bias adds stay fp32.
"""

import sys

if "/opt/trn_rl_repo" not in sys.path:
    sys.path.insert(0, "/opt/trn_rl_repo")

import ml_dtypes
import numpy as np

import concourse.bass as bass
import concourse.bacc as bacc
import concourse.mybir as mybir
import concourse.tile as tile
from concourse.bass_utils import run_bass_kernel_spmd
from concourse.masks import make_identity, make_upper_triangular

P = 128
T = 1024
C = 768
H = 12
D = 64
F = 3072
TT = T // P   # 8 token tiles
KC = C // P   # 6 feature tiles
NP = H // 2   # 6 head pairs
LN_EPS = 1e-5
f32 = mybir.dt.float32
bf16 = mybir.dt.bfloat16
AF = mybir.ActivationFunctionType
ALU = mybir.AluOpType

N_CORES = 8

WEIGHT_NAMES = [
    "ln1_g", "ln1_b", "w_attn", "b_attn", "w_proj", "b_proj",
    "ln2_g", "ln2_b", "w_fc1", "b_fc1", "w_fc2", "b_fc2",
]
BF16_NAMES = {"w_attn", "w_proj", "w_fc1", "w_fc2"}


def _layer_norm(nc, tmp, x_ap, g_b, b_b, out_h, eps_ap):
    """LN over the 768-wide free dim of a [128, 768] token tile (fp32)."""
    stats = tmp.tile([P, 3, 6], f32, tag="lnstats")
    xv = x_ap.rearrange("p (a b) -> p a b", b=256)
    for a in range(3):
        nc.vector.bn_stats(out=stats[:, a, :], in_=xv[:, a, :])
    mv = tmp.tile([P, 2], f32, tag="lnmv")
    nc.vector.bn_aggr(out=mv[:], in_=stats[:])
    rs = tmp.tile([P, 1], f32, tag="lnrs")
    nc.scalar.activation(out=rs[:], in_=mv[:, 1:2], func=AF.Sqrt,
                         bias=eps_ap, scale=1.0)
    rsr = tmp.tile([P, 1], f32, tag="lnrsr")
    nc.vector.reciprocal(out=rsr[:], in_=rs[:])
    h32 = tmp.tile([P, C], f32, tag="lnh32")
    nc.vector.tensor_scalar(out=h32[:], in0=x_ap, scalar1=mv[:, 0:1],
                            scalar2=rsr[:], op0=ALU.subtract, op1=ALU.mult)
    nc.vector.tensor_mul(out=h32[:], in0=h32[:], in1=g_b)
    nc.vector.tensor_add(out=out_h, in0=h32[:], in1=b_b)


def build_nc(sim_safe_gelu=False, debug_dump=False, n_copies=1):
    nc = bacc.Bacc("TRN2", target_bir_lowering=False, debug=False)

    x_d = nc.dram_tensor("x", [T, C], f32, kind="ExternalInput").ap()
    w = {}
    shapes = {
        "ln1_g": [C], "ln1_b": [C], "w_attn": [C, 3 * C], "b_attn": [3 * C],
        "w_proj": [C, C], "b_proj": [C], "ln2_g": [C], "ln2_b": [C],
        "w_fc1": [C, F], "b_fc1": [F], "w_fc2": [F, C], "b_fc2": [C],
    }
    for name in WEIGHT_NAMES:
        dt = bf16 if name in BF16_NAMES else f32
        w[name] = nc.dram_tensor(name, shapes[name], dt, kind="ExternalInput").ap()
    out_d = nc.dram_tensor("out", [T, C], f32, kind="ExternalOutput").ap()
    dbg = {}
    if debug_dump:
        for nm, shp, dt in [("dbg_h", [T, C], bf16), ("dbg_hT", [C, T], bf16),
                            ("dbg_qT", [C, T], bf16), ("dbg_kT", [C, T], bf16),
                            ("dbg_v", [T, H * (D + 1)], bf16),
                            ("dbg_att0", [T, T], bf16), ("dbg_YT", [C, T], bf16),
                            ("dbg_h2T", [C, T], bf16), ("dbg_R", [2 * H, T], f32)]:
            dbg[nm] = nc.dram_tensor(nm, shp, dt, kind="ExternalOutput").ap()

    with tile.TileContext(nc) as tc:
        for _rep in range(n_copies):
            with (
                tc.tile_pool(name="const", bufs=1) as cp,
                tc.tile_pool(name="YTp", bufs=1) as YTp,
            ):
                mask01 = cp.tile([P, P], bf16, tag="mask01")
                make_upper_triangular(nc, mask01[:], val=1.0, diag=True)
                epsc = cp.tile([P, 1], f32, tag="epsc")
                nc.vector.memset(epsc[:], LN_EPS)
                ones_c = cp.tile([P, D], bf16, tag="ones_c")
                nc.vector.memset(ones_c[:], 1.0)

                def bcast_const(name, src_ap):
                    t = cp.tile([P, C], f32, tag=name)
                    bc = bass.AP(tensor=src_ap.tensor, offset=src_ap.offset,
                                 ap=[[0, P]] + list(src_ap.ap))
                    nc.gpsimd.dma_start(out=t[:], in_=bc)
                    return t

                g1b = bcast_const("g1b", w["ln1_g"])
                b1b = bcast_const("b1b", w["ln1_b"])
                g2b = bcast_const("g2b", w["ln2_g"])
                b2b = bcast_const("b2b", w["ln2_b"])
                bvb = bcast_const("bvb", w["b_attn"][2 * C:3 * C])
                bpb = bcast_const("bpb", w["b_proj"])
                b2cb = bcast_const("b2cb", w["b_fc2"])

                bq = cp.tile([P, KC], f32, tag="bq")
                nc.sync.dma_start(out=bq[:], in_=w["b_attn"][0:C].rearrange("(m p) -> p m", p=P))
                bk = cp.tile([P, KC], f32, tag="bk")
                nc.sync.dma_start(out=bk[:], in_=w["b_attn"][C:2 * C].rearrange("(m p) -> p m", p=P))
                b1c = cp.tile([P, F // P], f32, tag="b1c")
                nc.sync.dma_start(out=b1c[:], in_=w["b_fc1"].rearrange("(m p) -> p m", p=P))

                YT = [YTp.tile([P, T], bf16, tag=f"YT{k}", name=f"YT{k}") for k in range(KC)]

                # ---------------- Phases 1-3 ----------------
                with (
                    tc.tile_pool(name="hTp", bufs=1) as hTp,
                    tc.tile_pool(name="vp", bufs=1) as vp,
                ):
                    hT = [hTp.tile([P, T], bf16, tag=f"hT{k}", name=f"hT{k}") for k in range(KC)]

                    # P1: LN1 + transpose
                    with (
                        tc.tile_pool(name="p1", bufs=3) as p1p,
                        tc.tile_pool(name="p1t", bufs=4) as p1t,
                    ):
                        for i in range(TT):
                            xt = p1p.tile([P, C], f32, tag="xt")
                            nc.sync.dma_start(out=xt[:], in_=x_d[i * P:(i + 1) * P, :])
                            h = p1p.tile([P, C], bf16, tag="h")
                            _layer_norm(nc, p1t, xt[:], g1b[:], b1b[:], h[:], epsc[:])
                            for k in range(KC):
                                nc.sync.dma_start_transpose(
                                    out=hT[k][:, i * P:(i + 1) * P],
                                    in_=h[:, k * P:(k + 1) * P])
                            if debug_dump:
                                nc.sync.dma_start(out=dbg["dbg_h"][i * P:(i + 1) * P, :], in_=h[:])
                        if debug_dump:
                            for k in range(KC):
                                nc.sync.dma_start(out=dbg["dbg_hT"][k * P:(k + 1) * P, :], in_=hT[k][:])

                    # P2a: V token-major, packed [v|1] per head
                    vts = []
                    with (
                        tc.tile_pool(name="wav", bufs=1) as wavp,
                        tc.tile_pool(name="psv", bufs=2, space="PSUM") as psv,
                    ):
                        wv = []
                        for k in range(KC):
                            wvt = wavp.tile([P, C], bf16, tag=f"wav{k}")
                            nc.sync.dma_start(out=wvt[:], in_=w["w_attn"][k * P:(k + 1) * P, 2 * C:3 * C])
                            wv.append(wvt)
                        for i in range(TT):
                            psvt = psv.tile([P, C], f32, tag="psv")
                            for k in range(KC):
                                lhsT = hT[k][:, i * P:(i + 1) * P]
                                nc.tensor.matmul(out=psvt[:, 0:512], lhsT=lhsT,
                                                 rhs=wv[k][:, 0:512],
                                                 start=(k == 0), stop=(k == KC - 1))
                                nc.tensor.matmul(out=psvt[:, 512:768], lhsT=lhsT,
                                                 rhs=wv[k][:, 512:768],
                                                 start=(k == 0), stop=(k == KC - 1))
                            vt = vp.tile([P, H * (D + 1)], bf16, tag=f"v{i}")
                            vv = vt[:].rearrange("p (h e) -> p h e", e=D + 1)
                            nc.vector.tensor_add(
                                out=vv[:, :, 0:D],
                                in0=psvt[:].rearrange("p (h e) -> p h e", e=D),
                                in1=bvb[:].rearrange("p (h e) -> p h e", e=D))
                            nc.vector.memset(vv[:, :, D:D + 1], 1.0)
                            vts.append(vt)
                            if debug_dump:
                                nc.sync.dma_start(out=dbg["dbg_v"][i * P:(i + 1) * P, :], in_=vt[:])

                    # P2b/P3: per head pair QK + attention
                    with (
                        tc.tile_pool(name="waqk", bufs=2) as waqkp,
                        tc.tile_pool(name="qk", bufs=2) as qkp,
                        tc.tile_pool(name="att", bufs=9) as attp,
                        tc.tile_pool(name="rsc", bufs=2) as rscp,
                        tc.tile_pool(name="yn", bufs=2) as ynp,
                        tc.tile_pool(name="psqs", bufs=3, space="PSUM") as psqs,
                        tc.tile_pool(name="psy", bufs=1, space="PSUM") as psyp,
                    ):
                        for pi in range(NP):
                            wq = waqkp.tile([P, KC, 2 * P], bf16, tag="waqk")
                            for k in range(KC):
                                nc.sync.dma_start(
                                    out=wq[:, k, 0:P],
                                    in_=w["w_attn"][k * P:(k + 1) * P, pi * P:(pi + 1) * P])
                                nc.sync.dma_start(
                                    out=wq[:, k, P:2 * P],
                                    in_=w["w_attn"][k * P:(k + 1) * P, C + pi * P:C + (pi + 1) * P])
                            qT = qkp.tile([P, T], bf16, tag="qT")
                            kT = qkp.tile([P, T], bf16, tag="kT")
                            for (dst, woff, bcol) in ((qT, 0, bq), (kT, P, bk)):
                                psq = psqs.tile([P, T], f32, tag="ps")
                                for k in range(KC):
                                    lhsT = wq[:, k, woff:woff + P]
                                    nc.tensor.matmul(out=psq[:, 0:512], lhsT=lhsT,
                                                     rhs=hT[k][:, 0:512],
                                                     start=(k == 0), stop=(k == KC - 1))
                                    nc.tensor.matmul(out=psq[:, 512:1024], lhsT=lhsT,
                                                     rhs=hT[k][:, 512:1024],
                                                     start=(k == 0), stop=(k == KC - 1))
                                nc.vector.tensor_scalar_add(out=dst[:], in0=psq[:],
                                                            scalar1=bcol[:, pi:pi + 1])
                            if debug_dump:
                                nc.sync.dma_start(out=dbg["dbg_qT"][pi * P:(pi + 1) * P, :], in_=qT[:])
                                nc.sync.dma_start(out=dbg["dbg_kT"][pi * P:(pi + 1) * P, :], in_=kT[:])

                            for hh in (2 * pi, 2 * pi + 1):
                                off = (hh % 2) * D
                                qh = qT[off:off + D, :]
                                kh = kT[off:off + D, :]
                                atts = []
                                for j in range(TT):
                                    nt = (TT - j) * P
                                    pss = psqs.tile([P, T], f32, tag="ps")
                                    for c0 in range(0, nt, 512):
                                        cw = min(512, nt - c0)
                                        nc.tensor.matmul(
                                            out=pss[:, c0:c0 + cw],
                                            lhsT=kh[:, j * P:(j + 1) * P],
                                            rhs=qh[:, j * P + c0:j * P + c0 + cw],
                                            start=True, stop=True)
                                    at = attp.tile([P, T], bf16, tag="att")
                                    nc.scalar.activation(out=at[:, 0:nt], in_=pss[:, 0:nt],
                                                         func=AF.Exp, scale=0.125)
                                    nc.vector.tensor_mul(out=at[:, 0:P], in0=at[:, 0:P],
                                                         in1=mask01[:])
                                    atts.append(at)
                                    if debug_dump and hh == 0:
                                        nc.sync.dma_start(
                                            out=dbg["dbg_att0"][j * P:(j + 1) * P, j * P:T],
                                            in_=at[:, 0:nt])
                                # att^T @ [v|1]: merged matmuls, one per (pass, j).
                                # Pass A covers t-tiles 0..3 (yA), pass B 4..7 (yB).
                                yA = psyp.tile([D + 1, 512], f32, tag="yA")
                                yB = psyp.tile([D + 1, 512], f32, tag="yB")
                                for j in range(4):
                                    vloc = vts[j][:, hh * (D + 1):(hh + 1) * (D + 1)]
                                    nc.tensor.matmul(
                                        out=yA[:, j * P:512], lhsT=vloc,
                                        rhs=atts[j][:, 0:(4 - j) * P],
                                        start=(j == 0), stop=(j == 3))
                                for j in range(TT):
                                    vloc = vts[j][:, hh * (D + 1):(hh + 1) * (D + 1)]
                                    c0 = max(j - 4, 0) * P
                                    r0 = (max(j, 4) - j) * P
                                    nc.tensor.matmul(
                                        out=yB[:, c0:512], lhsT=vloc,
                                        rhs=atts[j][:, r0:(TT - j) * P],
                                        start=(j == 0), stop=(j == TT - 1))
                                rrow = rscp.tile([D + 1, T], f32, tag="rrow")
                                nc.vector.reciprocal(out=rrow[D:D + 1, 0:512], in_=yA[D:D + 1, :])
                                nc.vector.reciprocal(out=rrow[D:D + 1, 512:1024], in_=yB[D:D + 1, :])
                                rbf = rscp.tile([D + 1, T], bf16, tag="rbf")
                                nc.vector.tensor_copy(out=rbf[D:D + 1, :], in_=rrow[D:D + 1, :])
                                psR = psqs.tile([P, T], f32, tag="ps", name="psR")
                                nc.tensor.matmul(out=psR[0:D, 0:512],
                                                 lhsT=ones_c[D:D + 1, :],
                                                 rhs=rbf[D:D + 1, 0:512],
                                                 start=True, stop=True)
                                nc.tensor.matmul(out=psR[0:D, 512:1024],
                                                 lhsT=ones_c[D:D + 1, :],
                                                 rhs=rbf[D:D + 1, 512:1024],
                                                 start=True, stop=True)
                                Rsb = rscp.tile([D, T], f32, tag="Rsb")
                                nc.scalar.copy(out=Rsb[:], in_=psR[0:D, :])
                                if debug_dump:
                                    nc.sync.dma_start(out=dbg["dbg_R"][2 * hh:2 * hh + 1, :],
                                                      in_=Rsb[0:1, :])
                                    nc.sync.dma_start(out=dbg["dbg_R"][2 * hh + 1:2 * hh + 2, :],
                                                      in_=Rsb[D - 1:D, :])
                                if off == 0:
                                    nc.vector.tensor_mul(out=YT[pi][0:D, 0:512],
                                                         in0=yA[0:D, :], in1=Rsb[:, 0:512])
                                    nc.vector.tensor_mul(out=YT[pi][0:D, 512:1024],
                                                         in0=yB[0:D, :], in1=Rsb[:, 512:1024])
                                else:
                                    ynt = ynp.tile([D, T], bf16, tag="yn")
                                    nc.vector.tensor_mul(out=ynt[:, 0:512],
                                                         in0=yA[0:D, :], in1=Rsb[:, 0:512])
                                    nc.vector.tensor_mul(out=ynt[:, 512:1024],
                                                         in0=yB[0:D, :], in1=Rsb[:, 512:1024])
                                    nc.sync.dma_start(out=YT[pi][D:P, :], in_=ynt[:])

                if debug_dump:
                    for k in range(KC):
                        nc.sync.dma_start(out=dbg["dbg_YT"][k * P:(k + 1) * P, :], in_=YT[k][:])

                # ---------------- Phases 4-6 ----------------
                with tc.tile_pool(name="h2Tp", bufs=1) as h2Tp:
                    h2T = [h2Tp.tile([P, T], bf16, tag=f"h2T{k}", name=f"h2T{k}") for k in range(KC)]
                    x2s = [h2Tp.tile([P, C], f32, tag=f"x2_{i}", name=f"x2_{i}") for i in range(TT)]

                    with (
                        tc.tile_pool(name="wpp", bufs=1) as wpp,
                        tc.tile_pool(name="p4", bufs=3) as p4p,
                        tc.tile_pool(name="p4t", bufs=4) as p4t,
                        tc.tile_pool(name="ps4", bufs=2, space="PSUM") as ps4,
                    ):
                        wps = []
                        for k in range(KC):
                            wpt = wpp.tile([P, C], bf16, tag=f"wp{k}")
                            nc.sync.dma_start(out=wpt[:], in_=w["w_proj"][k * P:(k + 1) * P, :])
                            wps.append(wpt)
                        for i in range(TT):
                            xre = p4p.tile([P, C], f32, tag="xre")
                            nc.sync.dma_start(out=xre[:], in_=x_d[i * P:(i + 1) * P, :])
                            psp = ps4.tile([P, C], f32, tag="psp")
                            for k in range(KC):
                                lhsT = YT[k][:, i * P:(i + 1) * P]
                                nc.tensor.matmul(out=psp[:, 0:512], lhsT=lhsT,
                                                 rhs=wps[k][:, 0:512],
                                                 start=(k == 0), stop=(k == KC - 1))
                                nc.tensor.matmul(out=psp[:, 512:768], lhsT=lhsT,
                                                 rhs=wps[k][:, 512:768],
                                                 start=(k == 0), stop=(k == KC - 1))
                            x2 = x2s[i]
                            nc.vector.tensor_add(out=x2[:], in0=psp[:], in1=xre[:])
                            nc.vector.tensor_add(out=x2[:], in0=x2[:], in1=bpb[:])
                            h2 = p4p.tile([P, C], bf16, tag="h2")
                            _layer_norm(nc, p4t, x2[:], g2b[:], b2b[:], h2[:], epsc[:])
                            for k in range(KC):
                                nc.sync.dma_start_transpose(
                                    out=h2T[k][:, i * P:(i + 1) * P],
                                    in_=h2[:, k * P:(k + 1) * P])
                        if debug_dump:
                            for k in range(KC):
                                nc.sync.dma_start(out=dbg["dbg_h2T"][k * P:(k + 1) * P, :], in_=h2T[k][:])

                    # P6: MLP in 512-wide hidden strips
                    with (
                        tc.tile_pool(name="mw", bufs=2) as mwp,
                        tc.tile_pool(name="gt", bufs=5) as gtp,
                        tc.tile_pool(name="accp", bufs=1) as accp,
                        tc.tile_pool(name="psg", bufs=2, space="PSUM") as psg,
                        tc.tile_pool(name="psf", bufs=2, space="PSUM") as psf,
                    ):
                        accs = [accp.tile([P, C], f32, tag=f"acc{i}", name=f"acc{i}") for i in range(TT)]
                        for s in range(F // 512):
                            w1s = []
                            for k in range(KC):
                                w1t = mwp.tile([P, 512], bf16, tag=f"w1_{k}")
                                nc.sync.dma_start(out=w1t[:],
                                                  in_=w["w_fc1"][k * P:(k + 1) * P, s * 512:(s + 1) * 512])
                                w1s.append(w1t)
                            w2s = []
                            for kk in range(4):
                                w2t = mwp.tile([P, C], bf16, tag=f"w2_{kk}")
                                nc.sync.dma_start(out=w2t[:],
                                                  in_=w["w_fc2"][(s * 4 + kk) * P:(s * 4 + kk + 1) * P, :])
                                w2s.append(w2t)
                            gts = []
                            for m in range(4):
                                psgt = psg.tile([P, T], f32, tag="psg")
                                for k in range(KC):
                                    lhsT = w1s[k][:, m * P:(m + 1) * P]
                                    nc.tensor.matmul(out=psgt[:, 0:512], lhsT=lhsT,
                                                     rhs=h2T[k][:, 0:512],
                                                     start=(k == 0), stop=(k == KC - 1))
                                    nc.tensor.matmul(out=psgt[:, 512:1024], lhsT=lhsT,
                                                     rhs=h2T[k][:, 512:1024],
                                                     start=(k == 0), stop=(k == KC - 1))
                                gt = gtp.tile([P, T], bf16, tag="gt")
                                if not sim_safe_gelu:
                                    nc.scalar.activation(out=gt[:], in_=psgt[:],
                                                         func=AF.Gelu_apprx_tanh,
                                                         bias=b1c[:, s * 4 + m:s * 4 + m + 1],
                                                         scale=1.0)
                                else:
                                    a = gtp.tile([P, T], f32, tag="ga", bufs=2)
                                    nc.scalar.activation(out=a[:], in_=psgt[:],
                                                         func=AF.Identity,
                                                         bias=b1c[:, s * 4 + m:s * 4 + m + 1],
                                                         scale=1.0)
                                    sq = gtp.tile([P, T], f32, tag="gsq", bufs=2)
                                    nc.scalar.activation(out=sq[:], in_=a[:], func=AF.Square)
                                    nc.scalar.activation(out=sq[:], in_=sq[:], func=AF.Identity,
                                                         bias=1.0, scale=0.044715)
                                    nc.vector.tensor_mul(out=sq[:], in0=sq[:], in1=a[:])
                                    nc.scalar.activation(out=sq[:], in_=sq[:], func=AF.Tanh,
                                                         scale=0.7978845608028654)
                                    nc.scalar.activation(out=sq[:], in_=sq[:], func=AF.Identity,
                                                         bias=1.0, scale=1.0)
                                    nc.vector.tensor_mul(out=gt[:], in0=sq[:], in1=a[:])
                                    nc.scalar.mul(out=gt[:], in_=gt[:], mul=0.5)
                                gts.append(gt)
                            for i in range(TT):
                                psft = psf.tile([P, C], f32, tag="psf")
                                for kk in range(4):
                                    lhsT = gts[kk][:, i * P:(i + 1) * P]
                                    nc.tensor.matmul(out=psft[:, 0:512], lhsT=lhsT,
                                                     rhs=w2s[kk][:, 0:512],
                                                     start=(kk == 0), stop=(kk == 3))
                                    nc.tensor.matmul(out=psft[:, 512:768], lhsT=lhsT,
                                                     rhs=w2s[kk][:, 512:768],
                                                     start=(kk == 0), stop=(kk == 3))
                                if s == 0:
                                    nc.vector.tensor_add(out=accs[i][:], in0=psft[:], in1=x2s[i][:])
                                    nc.vector.tensor_add(out=accs[i][:], in0=accs[i][:], in1=b2cb[:])
                                else:
                                    nc.vector.tensor_add(out=accs[i][:], in0=accs[i][:], in1=psft[:])
                                if s == F // 512 - 1:
                                    nc.sync.dma_start(out=out_d[i * P:(i + 1) * P, :], in_=accs[i][:])

    nc.compile()
    return nc


_NC_CACHE = {}


def _get_nc():
    if "nc" not in _NC_CACHE:
        _NC_CACHE["nc"] = build_nc()
    return _NC_CACHE["nc"]


def kernel(**inputs):
    x = np.ascontiguousarray(np.asarray(inputs["x"], dtype=np.float32))
    assert x.shape == (N_CORES, T, C), x.shape
    weights = {}
    for n in WEIGHT_NAMES:
        a = np.asarray(inputs[n], dtype=np.float32)
        if n in BF16_NAMES:
            a = a.astype(ml_dtypes.bfloat16)
        weights[n] = np.ascontiguousarray(a)
    nc = _get_nc()
    in_maps = []
    for c in range(N_CORES):
        m = {"x": np.ascontiguousarray(x[c])}
        m.update(weights)
        in_maps.append(m)
    res = run_bass_kernel_spmd(nc, in_maps, core_ids=list(range(N_CORES)))
    return np.stack([np.asarray(res.results[c]["out"]) for c in range(N_CORES)], axis=0)


if __name__ == "__main__":
    rng = np.random.default_rng(0)
    ins = {
        "x": rng.standard_normal((N_CORES, T, C), dtype=np.float32),
        "ln1_g": np.ones(C, np.float32), "ln1_b": np.zeros(C, np.float32),
        "w_attn": rng.standard_normal((C, 3 * C), dtype=np.float32) * 0.02,
        "b_attn": np.zeros(3 * C, np.float32),
        "w_proj": rng.standard_normal((C, C), dtype=np.float32) * 0.02,
        "b_proj": np.zeros(C, np.float32),
        "ln2_g": np.ones(C, np.float32), "ln2_b": np.zeros(C, np.float32),
        "w_fc1": rng.standard_normal((C, F), dtype=np.float32) * 0.02,
        "b_fc1": np.zeros(F, np.float32),
        "w_fc2": rng.standard_normal((F, C), dtype=np.float32) * 0.02,
        "b_fc2": np.zeros(C, np.float32),
    }
    out = kernel(**ins)
    print("out", out.shape, out.dtype, float(np.abs(out).max()))
